# revision 1
# baseline (speedup 1.0000x reference)
"""Trainium2 Bass kernel for nn_MIGAModel (moe_routing).

Strategy (pure data parallel over the stock axis N, 8 cores):
 - Host pre-transposes each core's x shard to xT [T*D, N/8] and splits
   it into an fp16 hi/lo pair, interleaved per element (same 4
   bytes/elem of DMA traffic as fp32, but fp16 matmuls run at 1
   cycle/row on the PE vs 4 for fp32, and the kernel is DMA-bound).
   Router: h = xh@Wh + xh@Wl + xl@Wh accumulated in fp32 PSUM -- the
   fp16 pair represents x and Wr to ~2^-22 relative, so the exact
   top-2 routing decisions survive (HW rel err ~5e-4 end to end; the
   bf16 pair loses ~2^-17 and flips a few near-tied routes).
 - Row-chunk-outer / K-inner pipeline: each chunk (<=512 rows, one
   PSUM bank) streams its K-tiles in small DMA groups while the
   PREVIOUS chunk's gating/attention post-processing overlaps the
   stream; the only serial tail is the 128-row last chunk's post work.
 - The expert layer is composed into the Q/K/V projection matrices on
   the host (AQc = WET @ AQ etc., be folded into their biases); the
   output projection + m1 weighting is folded into w = AO @ m1 so the
   final combine is short. Attention matmuls run in fp16; only the
   top-2 selection needs exactness and that path stays fp32.
 - Exact top-2 gating: PE transposes of fp32 h, batched free-axis
   reduce_max twice, mask built in transposed space and PE-transposed
   back, weighted sum via ones-matmuls.
"""
import sys
import numpy as np

for _p in ("/opt/trn_rl_repo",):
    if _p not in sys.path:
        sys.path.insert(0, _p)

import concourse.bass as bass
import concourse.tile as tile
from concourse import bacc, mybir
from concourse.bass_utils import run_bass_kernel_spmd

F32 = mybir.dt.float32
BF16 = mybir.dt.bfloat16
F16 = mybir.dt.float16
ROUTER_DT = "fp16"            # "bf16" or "fp16" hi/lo pair
POST_DT = "fp16"

N, T, D = 20000, 60, 158
TD = T * D                      # 9480
G, E, H, DH, GE = 8, 16, 4, 4, 128
NCORES = 8
NSH = N // NCORES               # 2500 rows per core
KT = TD // 128                  # 74 full K-tiles
KTAIL = TD - KT * 128           # 8 leftover K rows
KG = 3                          # K-tiles per DMA group
CHUNKS = [512, 512, 512, 512, 324, 128]     # sum == NSH; small tail chunk
assert sum(CHUNKS) == NSH
assert all(w * 4 >= 512 for w in CHUNKS)    # paired DMA descriptor >= 512B

# packed matrix indices (each a [128,128] block in the "mats" input)
M_AQ = 0
M_AK0 = 1                       # 4
M_AV0 = 5                       # 4
M_MS0 = 9                       # 4
M_MER0 = 13                     # 4
M_MDEN = 17
M_AOT = 18
M_IDT = 19
M_ONES = 20
NMATS = 21

# bias pack columns
B_BR, B_BQ, B_BK0, B_BV0, B_BO = 0, 1, 2, 6, 10
NBIAS = 16


def build_consts(Wr, br, We, be, Wq, bq, Wk, bk, Wv, bv, Wo, bo):
    """Host-side packed constants. Returns (wr [TD,GE], mats, biasp)."""
    f32 = np.float32
    Wr = np.asarray(Wr, f32)
    br = np.asarray(br, f32)
    We = np.asarray(We, f32)
    be = np.asarray(be, f32)
    Wq = np.asarray(Wq, f32)
    bq = np.asarray(bq, f32)
    Wk = np.asarray(Wk, f32)
    bk = np.asarray(bk, f32)
    Wv = np.asarray(Wv, f32)
    bv = np.asarray(bv, f32)
    Wo = np.asarray(Wo, f32)
    bo = np.asarray(bo, f32)

    # base packed blocks (as matmul lhsT: result = block^T @ input)
    WET = np.transpose(We, (2, 0, 1)).reshape(GE, GE).astype(f32)
    AQ = np.zeros((GE, GE), f32)
    AK = np.zeros((DH, GE, GE), f32)
    AV = np.zeros((DH, GE, GE), f32)
    bq_p = np.zeros(GE, f32)
    bk_p = np.zeros((DH, GE), f32)
    bv_p = np.zeros((DH, GE), f32)
    d_ = np.arange(DH)
    for g in range(G):
        for h in range(H):
            for d in range(DH):
                p = d * 32 + g * 4 + h
                AQ[g * 16:(g + 1) * 16, p] = Wq[g, h * 4 + d, :]
                bq_p[p] = bq[g, h * 4 + d]
            for e in range(DH):
                ps = d_ * 32 + g * 4 + h
                for p in ps:
                    AK[e, g * 16:(g + 1) * 16, p] = Wk[g, h * 4 + e, :]
                    AV[e, g * 16:(g + 1) * 16, p] = Wv[g, h * 4 + e, :]
                    bk_p[e, p] = bk[g, h * 4 + e]
                    bv_p[e, p] = bv[g, h * 4 + e]

    mats = np.zeros((NMATS, GE, GE), f32)
    biasp = np.zeros((GE, NBIAS), f32)

    be_v = be.reshape(GE)
    mats[M_AQ] = WET @ AQ
    biasp[:, B_BQ] = AQ.T @ be_v + bq_p
    for e in range(DH):
        mats[M_AK0 + e] = WET @ AK[e]
        biasp[:, B_BK0 + e] = AK[e].T @ be_v + bk_p[e]
        mats[M_AV0 + e] = WET @ AV[e]
        biasp[:, B_BV0 + e] = AV[e].T @ be_v + bv_p[e]
    for e in range(DH):
        for d in range(DH):
            for g in range(G):
                for h in range(H):
                    mats[M_MS0 + e, d * 32 + g * 4 + h, e * 32 + d * 8 + g] = 1.0
                    mats[M_MDEN, e * 32 + d * 8 + g, d * 32 + g * 4 + h] = 1.0
                    mats[M_MER0 + e, e * 32 + d * 8 + g, d * 32 + g * 4 + h] = 1.0
    for g in range(G):
        for f in range(E):
            for h in range(H):
                for d in range(DH):
                    # stored transposed: w = AO @ m1 (for the nums fold)
                    mats[M_AOT, g * 16 + f, d * 32 + g * 4 + h] = Wo[g, f, h * 4 + d]
    mats[M_IDT] = np.eye(GE, dtype=f32)
    mats[M_ONES] = 1.0

    biasp[:, B_BR] = br
    biasp[:, B_BO] = bo.reshape(GE)

    # [128, NMATS*128] column-packed
    mats_packed = np.ascontiguousarray(
        np.transpose(mats, (1, 0, 2)).reshape(GE, NMATS * GE))
    return Wr, mats_packed, biasp


def core_inputs(x, Wr, br, We, be, Wq, bq, Wk, bk, Wv, bv, Wo, bo):
    """Host prep: returns the per-core in_map list."""
    import ml_dtypes
    f32 = np.float32
    bf = ml_dtypes.bfloat16
    x = np.asarray(x, f32)
    wr, mats_packed, biasp = build_consts(
        Wr, br, We, be, Wq, bq, Wk, bk, Wv, bv, Wo, bo)

    rdt = bf if ROUTER_DT == "bf16" else np.float16
    pdt = bf if POST_DT == "bf16" else np.float16
    # wr hi/lo interleaved [TD, 2, GE] so DMA descriptors are 512B
    wh = wr.astype(rdt)
    wl = (wr - wh.astype(f32)).astype(rdt)
    wp = np.ascontiguousarray(
        np.stack([wh, wl], axis=1))               # [TD, 2, GE]
    matsb = mats_packed.astype(pdt)
    biasb = biasp.astype(pdt)

    in_maps = []
    for c in range(NCORES):
        xs = x[c * NSH:(c + 1) * NSH].reshape(NSH, TD)
        xt = np.ascontiguousarray(xs.T)           # [TD, NSH] fp32
        xh = xt.astype(rdt)
        xl = (xt - xh.astype(f32)).astype(rdt)
        xp = np.ascontiguousarray(
            np.stack([xh, xl], axis=-1))          # [TD, NSH, 2]
        in_maps.append({"xp": xp, "wp": wp,
                        "mats": mats_packed, "matsb": matsb,
                        "bias": biasp, "biasb": biasb})
    return in_maps


def build_kernel():
    """Trace the Bass/Tile kernel; returns the compiled Bacc."""
    nc = bacc.Bacc("TRN2", target_bir_lowering=False, debug=False,
                   num_devices=NCORES)

    RDT = BF16 if ROUTER_DT == "bf16" else F16
    PDT = BF16 if POST_DT == "bf16" else F16
    xp_d = nc.dram_tensor("xp", [TD, NSH, 2], RDT, kind="ExternalInput").ap()
    wp_d = nc.dram_tensor("wp", [TD, 2, GE], RDT, kind="ExternalInput").ap()
    mats_d = nc.dram_tensor("mats", [GE, NMATS * GE], F32, kind="ExternalInput").ap()
    matsb_d = nc.dram_tensor("matsb", [GE, NMATS * GE], PDT, kind="ExternalInput").ap()
    bias_d = nc.dram_tensor("bias", [GE, NBIAS], F32, kind="ExternalInput").ap()
    biasb_d = nc.dram_tensor("biasb", [GE, NBIAS], PDT, kind="ExternalInput").ap()
    out_d = nc.dram_tensor("out", [1, NSH], F32, kind="ExternalOutput").ap()

    # K-groups: (tile_start, n_full_tiles, has_tail)
    def make_groups(kg):
        groups = []
        t = 0
        while t < KT:
            n = min(kg, KT - t)
            groups.append([t, n, False])
            t += n
        groups[-1][2] = True  # tail rides with the last group
        return groups

    with tile.TileContext(nc) as tc:
        with (
            tc.tile_pool(name="consts", bufs=1) as consts,
            tc.tile_pool(name="xts", bufs=10) as xts,
            tc.tile_pool(name="work", bufs=1) as work,
            tc.tile_pool(name="ps", bufs=2, space="PSUM") as ptp,
        ):
            # ---- constants to SBUF (DMAs deferred off the hot start) ----
            idt_sb = consts.tile([GE, GE], F32, tag="idt")
            matsb_sb = consts.tile([GE, NMATS * GE], PDT, tag="matsb")
            bias_sb = consts.tile([GE, NBIAS], F32, tag="bias")
            biasb_sb = consts.tile([GE, NBIAS], PDT, tag="biasb")
            pred_sb = consts.tile([1, NSH], F32, tag="pred")

            def load_consts():
                nc.sync.dma_start(out=idt_sb,
                                  in_=mats_d[:, M_IDT * GE:(M_IDT + 1) * GE])
                nc.sync.dma_start(out=matsb_sb, in_=matsb_d)
                nc.sync.dma_start(out=bias_sb, in_=bias_d)
                nc.sync.dma_start(out=biasb_sb, in_=biasb_d)

            wp_sb = consts.tile([128, KT, 2, GE], RDT, tag="wp")
            wpt_sb = consts.tile([KTAIL, 2, GE], RDT, tag="wpt")

            def matb(i):
                return matsb_sb[:, i * GE:(i + 1) * GE]

            def bcol(i):
                return bias_sb[:, i:i + 1]

            idt = idt_sb

            def load_wp_group(t0, n):
                nc.sync.dma_start(
                    out=wp_sb[:, t0:t0 + n, :, :],
                    in_=wp_d[t0 * 128:(t0 + n) * 128, :, :]
                        .rearrange("(t p) u m -> p t u m", p=128))

            # ---------------- per-chunk post-processing ----------------
            def post(c, c0, W, rtr, sfx=""):
                cn = f"c{c}{sfx}"
                # h = router psum + br (fp32 for the gating path, bf16
                # copy for the attention matmuls)
                h_sb = work.tile([GE, W], F32, tag="h" + sfx, name=f"h_{cn}")
                nc.scalar.activation(h_sb, rtr[:, :],
                                     mybir.ActivationFunctionType.Identity,
                                     bias=bcol(B_BR), scale=1.0)
                h16 = work.tile([GE, W], PDT, tag="h16" + sfx, name=f"h16_{cn}")
                nc.scalar.activation(h16, rtr[:, :],
                                     mybir.ActivationFunctionType.Identity,
                                     bias=bcol(B_BR), scale=1.0)

                # --- exact top-2 threshold (second max per row) ---
                nb = (W + 127) // 128
                trp = ptp.tile([128, 4, GE], F32, tag="scr", name=f"trp_{cn}")
                cs_last = W - (nb - 1) * 128
                if cs_last < 128:
                    # partition window must start at a multiple of 32; the
                    # transpose below overwrites the valid overlap
                    ms0 = (cs_last // 32) * 32
                    nc.vector.memset(trp[ms0:128, nb - 1, :], -1e30)
                for b in range(nb):
                    off = b * 128
                    cs = min(128, W - off)
                    nc.tensor.transpose(trp[:cs, b, :], h_sb[:, off:off + cs],
                                        idt)
                mx1 = work.tile([128, 4], F32, tag="mx1" + sfx, name=f"mx1_{cn}")
                nc.vector.reduce_max(mx1[:, :nb], trp[:, :nb, :],
                                     axis=mybir.AxisListType.X)
                mx2 = work.tile([128, 4], F32, tag="mx2" + sfx, name=f"mx2_{cn}")
                mx1b = mx1[:, :nb].unsqueeze(2).to_broadcast([128, nb, GE])
                eqm = work.tile([128, 4, GE], F32, tag="eqm" + sfx, name=f"eqm_{cn}")
                nc.vector.tensor_tensor(eqm[:, :nb, :], trp[:, :nb, :], mx1b,
                                        op=mybir.AluOpType.is_ge)
                hm = work.tile([128, 4, GE], F32, tag="hm" + sfx, name=f"hm_{cn}")
                nc.vector.scalar_tensor_tensor(
                    hm[:, :nb, :], in0=eqm[:, :nb, :], scalar=-1e30,
                    in1=trp[:, :nb, :],
                    op0=mybir.AluOpType.mult, op1=mybir.AluOpType.add)
                nc.vector.reduce_max(mx2[:, :nb], hm[:, :nb, :],
                                     axis=mybir.AxisListType.X)
                # mask in transposed space, then PE-transpose back
                mx2b = mx2[:, :nb].unsqueeze(2).to_broadcast([128, nb, GE])
                mkt = work.tile([128, 4, GE], F32, tag="mkt" + sfx, name=f"mkt_{cn}")
                nc.vector.tensor_tensor(mkt[:, :nb, :], trp[:, :nb, :], mx2b,
                                        op=mybir.AluOpType.is_ge)
                mask_ps = ptp.tile([GE, W], F32, tag="pp", bufs=3, name=f"mask_{cn}")
                for b in range(nb):
                    off = b * 128
                    cs = min(128, W - off)
                    nc.tensor.transpose(mask_ps[:, off:off + cs],
                                        mkt[:cs, b, :], idt[:cs, :cs])
                eh = work.tile([GE, W], F32, tag="eh" + sfx, name=f"eh_{cn}")
                nc.scalar.activation(eh, rtr[:, :],
                                     mybir.ActivationFunctionType.Exp,
                                     bias=bcol(B_BR), scale=1.0)
                m1 = work.tile([GE, W], PDT, tag="m1" + sfx, name=f"m1_{cn}")
                nc.vector.tensor_mul(m1, eh, mask_ps[:, :])
                # w = AO @ m1 (off the critical chain; folds the output
                # projection + m1 weighting of the final sum)
                w_ps = ptp.tile([GE, W], F32, tag="pp", bufs=3, name=f"w_{cn}")
                nc.tensor.matmul(w_ps[:, :], lhsT=matb(M_AOT), rhs=m1,
                                 start=True, stop=True)

                # --- attention (expert layer composed into Q/K/V) ---
                q_ps = ptp.tile([GE, W], F32, tag="pp", bufs=3, name=f"q_{cn}")
                nc.tensor.matmul(q_ps[:, :], lhsT=matb(M_AQ), rhs=h16,
                                 start=True, stop=True)
                qt = work.tile([GE, W], F32, tag="qt" + sfx, name=f"qt_{cn}")
                nc.scalar.activation(qt, q_ps[:, :],
                                     mybir.ActivationFunctionType.Identity,
                                     bias=bcol(B_BQ), scale=1.0)

                sc_ps = ptp.tile([GE, W], F32, tag="scr", name=f"sc_{cn}")
                for e in range(DH):
                    kr_ps = ptp.tile([GE, W], F32, tag="pp", bufs=3, name=f"kr_{cn}_{e}")
                    nc.tensor.matmul(kr_ps[:, :], lhsT=matb(M_AK0 + e),
                                     rhs=h16, start=True, stop=True)
                    pe_sb = work.tile([GE, W], PDT, tag="pe" + sfx, bufs=2,
                                      name=f"pe_{cn}_{e}")
                    nc.vector.scalar_tensor_tensor(
                        pe_sb, in0=kr_ps[:, :], scalar=bcol(B_BK0 + e), in1=qt,
                        op0=mybir.AluOpType.add, op1=mybir.AluOpType.mult)
                    nc.tensor.matmul(sc_ps[:, :], lhsT=matb(M_MS0 + e),
                                     rhs=pe_sb,
                                     start=(e == 0), stop=(e == DH - 1))
                es = work.tile([GE, W], PDT, tag="es" + sfx, name=f"es_{cn}")
                nc.scalar.activation(es, sc_ps[:, :],
                                     mybir.ActivationFunctionType.Exp,
                                     scale=0.5)

                den_ps = ptp.tile([GE, W], F32, tag="pp", bufs=3, name=f"den_{cn}")
                nc.tensor.matmul(den_ps[:, :], lhsT=matb(M_MDEN), rhs=es,
                                 start=True, stop=True)
                drec = work.tile([GE, W], F32, tag="drec" + sfx, name=f"drec_{cn}")
                nc.vector.reciprocal(drec, den_ps[:, :])
                wd = work.tile([GE, W], F32, tag="wd" + sfx, name=f"wd_{cn}")
                nc.vector.tensor_mul(wd, w_ps[:, :], drec)

                # att = sum_e er_e * (vr_e + bv_e), DVE tree reduction
                prods = []
                for e in range(DH):
                    vr_ps = ptp.tile([GE, W], F32, tag="pp", bufs=3, name=f"vr_{cn}_{e}")
                    nc.tensor.matmul(vr_ps[:, :], lhsT=matb(M_AV0 + e),
                                     rhs=h16, start=True, stop=True)
                    vrb = work.tile([GE, W], F32, tag=f"vrb{e % 2}",
                                    name=f"vrb_{cn}_{e}")
                    nc.scalar.activation(vrb, vr_ps[:, :],
                                         mybir.ActivationFunctionType.Identity,
                                         bias=bcol(B_BV0 + e), scale=1.0)
                    er_ps = ptp.tile([GE, W], F32, tag="pp", bufs=3, name=f"er_{cn}_{e}")
                    nc.tensor.matmul(er_ps[:, :], lhsT=matb(M_MER0 + e),
                                     rhs=es, start=True, stop=True)
                    pr = work.tile([GE, W], F32, tag=f"pr{e % 2}",
                                   name=f"pr_{cn}_{e}")
                    nc.vector.tensor_mul(pr, er_ps[:, :], vrb)
                    prods.append(pr)
                t01 = work.tile([GE, W], F32, tag="t01" + sfx, name=f"t01_{cn}")
                nc.vector.tensor_add(t01, prods[0], prods[1])
                t23 = work.tile([GE, W], F32, tag="t23" + sfx, name=f"t23_{cn}")
                nc.vector.tensor_add(t23, prods[2], prods[3])
                att = work.tile([GE, W], F32, tag="att" + sfx, name=f"att_{cn}")
                nc.vector.tensor_add(att, t01, t23)
                num = work.tile([GE, W], PDT, tag="num" + sfx, name=f"num_{cn}")
                nc.vector.tensor_mul(num, att, wd)

                # --- weighted combine (bo folded via bo^T @ m1) ---
                nd_ps = ptp.tile([33, W], F32, tag="nd", bufs=1,
                                 name=f"nd_{cn}")
                nc.tensor.matmul(nd_ps[0:1, :],
                                 lhsT=matsb_sb[:, M_ONES * GE:M_ONES * GE + 1],
                                 rhs=num, start=True, stop=False)
                nc.tensor.matmul(nd_ps[0:1, :],
                                 lhsT=biasb_sb[:, B_BO:B_BO + 1],
                                 rhs=m1, start=False, stop=True)
                nc.tensor.matmul(nd_ps[32:33, :],
                                 lhsT=matsb_sb[:, M_ONES * GE:M_ONES * GE + 1],
                                 rhs=m1, start=True, stop=True)
                rden = work.tile([1, W], F32, tag="rden" + sfx, name=f"rden_{cn}")
                nc.vector.reciprocal(rden, nd_ps[32:33, :])
                nc.vector.tensor_mul(pred_sb[0:1, c0:c0 + W],
                                     nd_ps[0:1, :], rden)

            # ---------------- main pipeline ----------------
            c0 = 0
            last = len(CHUNKS) - 1
            for c, W in enumerate(CHUNKS):
                rtr = ptp.tile([GE, W], F32, tag="rtr", bufs=2,
                               name=f"rtr_c{c}")
                first = True
                groups = make_groups(KG)
                for gi, (t0, ntl, tail) in enumerate(groups):
                    if c == 0 and gi == 1:
                        load_consts()
                    if c == 0:
                        load_wp_group(t0, ntl)
                        if tail:
                            nc.sync.dma_start(out=wpt_sb,
                                              in_=wp_d[KT * 128:TD, :, :])
                    xb = xts.tile([128, KG + 1, W, 2], RDT, tag="xb",
                                  name=f"xb_c{c}_g{gi}")
                    nc.sync.dma_start(
                        out=xb[:, :ntl, :, :],
                        in_=xp_d[t0 * 128:(t0 + ntl) * 128, c0:c0 + W, :]
                            .rearrange("(t p) m u -> p t m u", p=128))
                    if tail:
                        nc.sync.dma_start(
                            out=xb[:KTAIL, ntl, :, :],
                            in_=xp_d[KT * 128:TD, c0:c0 + W, :])
                    for i in range(ntl):
                        t = t0 + i
                        nc.tensor.matmul(rtr[:, :], lhsT=wp_sb[:, t, 0, :],
                                         rhs=xb[:, i, :, 0],
                                         start=first, stop=False)
                        first = False
                        nc.tensor.matmul(rtr[:, :], lhsT=wp_sb[:, t, 1, :],
                                         rhs=xb[:, i, :, 0],
                                         start=False, stop=False)
                        nc.tensor.matmul(rtr[:, :], lhsT=wp_sb[:, t, 0, :],
                                         rhs=xb[:, i, :, 1],
                                         start=False, stop=False)
                    if tail:
                        nc.tensor.matmul(rtr[:, :], lhsT=wpt_sb[:, 0, :],
                                         rhs=xb[:KTAIL, ntl, :, 0],
                                         start=False, stop=False)
                        nc.tensor.matmul(rtr[:, :], lhsT=wpt_sb[:, 1, :],
                                         rhs=xb[:KTAIL, ntl, :, 0],
                                         start=False, stop=False)
                        nc.tensor.matmul(rtr[:, :], lhsT=wpt_sb[:, 0, :],
                                         rhs=xb[:KTAIL, ntl, :, 1],
                                         start=False, stop=True)
                post(c, c0, W, rtr)
                c0 += W
            nc.sync.dma_start(out=out_d, in_=pred_sb)

    nc.compile()
    return nc


_NC_CACHE = None
LAST_RESULTS = None


def kernel(x, Wr, br, We, be, Wq, bq, Wk, bk, Wv, bv, Wo, bo):
    global _NC_CACHE, LAST_RESULTS
    if _NC_CACHE is None:
        _NC_CACHE = build_kernel()
    nc = _NC_CACHE

    in_maps = core_inputs(x, Wr, br, We, be, Wq, bq, Wk, bk, Wv, bv, Wo, bo)
    res = run_bass_kernel_spmd(nc, in_maps, list(range(NCORES)))
    LAST_RESULTS = res
    out = np.concatenate([res.results[c]["out"].reshape(NSH)
                          for c in range(NCORES)])
    return out.astype(np.float32)



# revision 2
# speedup vs baseline: 1.2459x; 1.2459x over previous
"""Trainium2 Bass kernel for nn_MIGAModel (moe_routing) — flag-and-fix v2.

Strategy (pure data parallel over the stock axis N, 8 cores):
 - Stream x as SINGLE fp16 [TD, NSHP] (2 B/elem — half the baseline's DMA,
   which was the binding roofline). Router pass: h_cheap = wh16^T @ xh16.
 - Exact top-2 routing survives because rows whose 2nd/3rd-max gap is
   below TAU (~6 sigma of the fp16 h-noise) are FLAGGED and recomputed
   exactly: a triangular-matmul prefix-sum compacts the flagged row
   indices, dma_gather pulls their raw fp32 rows, and a fp16 hi/lo pair
   matmul (2^-22 accurate) rebuilds h for just those rows (<=16/chunk).
   The fixed predictions overwrite the cheap ones via a onehot-matmul
   scatter + copy_predicated blend.
 - Attention/gating post-processing identical to the proven baseline
   (expert layer composed into Q/K/V, output proj folded into w = AO@m1).
 - N rows per core padded 2500 -> 2560; chunks [512x4, 384, 128]. The last
   128 columns stream as an exact fp16 hi/lo pair (3-matmul trick) so the
   final chunk needs no flag/gather/fix, letting the combined fix-post for
   all other chunks hide under its stream.
"""
import sys
import numpy as np

for _p in ("/opt/trn_rl_repo",):
    if _p not in sys.path:
        sys.path.insert(0, _p)

import concourse.bass as bass
import concourse.tile as tile
from concourse import bacc, mybir
from concourse.bass_utils import run_bass_kernel_spmd
from concourse.library_config import mlp

F32 = mybir.dt.float32
F16 = mybir.dt.float16
I16 = mybir.dt.int16
U8 = mybir.dt.uint8
PDT = F16
AX = mybir.AxisListType.X
OP = mybir.AluOpType
AF = mybir.ActivationFunctionType

N, T, D = 20000, 60, 158
TD = T * D                      # 9480
G, E, H, DH, GE = 8, 16, 4, 4, 128
NCORES = 8
NSH = N // NCORES               # 2500
NSHP = 2560                     # padded rows/core
CHUNKS = [512, 512, 512, 512, 384, 128]
WEX = 128                       # last chunk: exact fp16-pair stream, no fix
WMAX = 512
NB = WMAX // 128                # max blocks per chunk
NBLK = NSHP // 128              # 20
KT = TD // 128                  # 74
KTAIL = TD - KT * 128           # 8
KG = 3                          # K-tiles per stream DMA group
TDP = 9600                      # padded K for gather/fix (75*128)
KB32 = TDP // 128               # 75
S = 16                          # fix slots per chunk
TAU = 2e-3
FBATCH = [32, 32, 11]           # fix K-block batches (sum == KB32)

# packed matrix indices (each a [128,128] block in the "mats" input)
M_AQ = 0
M_AK0 = 1
M_AV0 = 5
M_MS0 = 9
M_MER0 = 13
M_MDEN = 17
M_AOT = 18
M_IDT = 19
M_ONES = 20
NMATS = 21

B_BR, B_BQ, B_BK0, B_BV0, B_BO = 0, 1, 2, 6, 10
NBIAS = 16

# fixc const columns
FC_J = 0
FC_G = S
FC_PM = S + NBLK
NFIXC = S + 2 * NBLK


def build_consts(Wr, br, We, be, Wq, bq, Wk, bk, Wv, bv, Wo, bo):
    """Host-side packed constants. Returns (wr [TD,GE], mats, biasp)."""
    f32 = np.float32
    Wr = np.asarray(Wr, f32)
    br = np.asarray(br, f32)
    We = np.asarray(We, f32)
    be = np.asarray(be, f32)
    Wq = np.asarray(Wq, f32)
    bq = np.asarray(bq, f32)
    Wk = np.asarray(Wk, f32)
    bk = np.asarray(bk, f32)
    Wv = np.asarray(Wv, f32)
    bv = np.asarray(bv, f32)
    Wo = np.asarray(Wo, f32)
    bo = np.asarray(bo, f32)

    WET = np.transpose(We, (2, 0, 1)).reshape(GE, GE).astype(f32)
    AQ = np.zeros((GE, GE), f32)
    AK = np.zeros((DH, GE, GE), f32)
    AV = np.zeros((DH, GE, GE), f32)
    bq_p = np.zeros(GE, f32)
    bk_p = np.zeros((DH, GE), f32)
    bv_p = np.zeros((DH, GE), f32)
    d_ = np.arange(DH)
    for g in range(G):
        for h in range(H):
            for d in range(DH):
                p = d * 32 + g * 4 + h
                AQ[g * 16:(g + 1) * 16, p] = Wq[g, h * 4 + d, :]
                bq_p[p] = bq[g, h * 4 + d]
            for e in range(DH):
                ps = d_ * 32 + g * 4 + h
                for p in ps:
                    AK[e, g * 16:(g + 1) * 16, p] = Wk[g, h * 4 + e, :]
                    AV[e, g * 16:(g + 1) * 16, p] = Wv[g, h * 4 + e, :]
                    bk_p[e, p] = bk[g, h * 4 + e]
                    bv_p[e, p] = bv[g, h * 4 + e]

    mats = np.zeros((NMATS, GE, GE), f32)
    biasp = np.zeros((GE, NBIAS), f32)

    be_v = be.reshape(GE)
    mats[M_AQ] = WET @ AQ
    biasp[:, B_BQ] = AQ.T @ be_v + bq_p
    for e in range(DH):
        mats[M_AK0 + e] = WET @ AK[e]
        biasp[:, B_BK0 + e] = AK[e].T @ be_v + bk_p[e]
        mats[M_AV0 + e] = WET @ AV[e]
        biasp[:, B_BV0 + e] = AV[e].T @ be_v + bv_p[e]
    for e in range(DH):
        for d in range(DH):
            for g in range(G):
                for h in range(H):
                    mats[M_MS0 + e, d * 32 + g * 4 + h, e * 32 + d * 8 + g] = 1.0
                    mats[M_MDEN, e * 32 + d * 8 + g, d * 32 + g * 4 + h] = 1.0
                    mats[M_MER0 + e, e * 32 + d * 8 + g, d * 32 + g * 4 + h] = 1.0
    for g in range(G):
        for f in range(E):
            for h in range(H):
                for d in range(DH):
                    mats[M_AOT, g * 16 + f, d * 32 + g * 4 + h] = Wo[g, f, h * 4 + d]
    mats[M_IDT] = np.eye(GE, dtype=f32)
    mats[M_ONES] = 1.0

    biasp[:, B_BR] = br
    biasp[:, B_BO] = bo.reshape(GE)

    mats_packed = np.ascontiguousarray(
        np.transpose(mats, (1, 0, 2)).reshape(GE, NMATS * GE))
    return Wr, mats_packed, biasp


def core_inputs(x, Wr, br, We, be, Wq, bq, Wk, bk, Wv, bv, Wo, bo):
    """Host prep: returns the per-core in_map list."""
    f32 = np.float32
    f16 = np.float16
    x = np.asarray(x, f32)
    wr, mats_packed, biasp = build_consts(
        Wr, br, We, be, Wq, bq, Wk, bk, Wv, bv, Wo, bo)

    wh = wr.astype(f16)
    wl = (wr - wh.astype(f32)).astype(f16)
    whl = np.zeros((TDP, 2, GE), f16)
    whl[:TD, 0] = wh
    whl[:TD, 1] = wl
    matsb = mats_packed.astype(f16)
    biasb = biasp.astype(f16)
    ltu = np.triu(np.ones((128, 128), f32), 1).astype(f16)
    fixc = np.zeros((128, NFIXC), f32)
    fixc[:, FC_J:FC_J + S] = np.arange(S)[None, :]
    p128 = np.arange(128)
    for g_ in range(NBLK):
        fixc[:, FC_G + g_] = p128 + 128 * g_
        fixc[:, FC_PM + g_] = (p128 + 128 * g_ < NSH).astype(f32)

    in_maps = []
    ex0 = NSHP - WEX                # first column of the exact chunk
    for c in range(NCORES):
        xs = x[c * NSH:(c + 1) * NSH].reshape(NSH, TD)
        xt = np.ascontiguousarray(xs.T)
        xh = xt[:, :NSHP - WEX].astype(f16)
        xe32 = np.zeros((TD, WEX), f32)
        ncols = max(0, NSH - ex0)
        if ncols:
            xe32[:, :ncols] = xt[:, ex0:NSH]
        xeh = xe32.astype(f16)
        xel = (xe32 - xeh.astype(f32)).astype(f16)
        xp5 = np.ascontiguousarray(
            np.stack([xeh, xel], axis=-1))          # [TD, WEX, 2]
        x32 = np.zeros((NSH, TDP), f32)
        x32[:, :TD] = xs
        in_maps.append({"xh": xh, "x32": x32, "whl": whl, "xp5": xp5,
                        "mats": mats_packed, "matsb": matsb,
                        "bias": biasp, "biasb": biasb,
                        "ltu": ltu, "fixc": fixc})
    return in_maps


def build_kernel():
    """Trace the Bass/Tile kernel; returns the compiled Bacc."""
    nc = bacc.Bacc("TRN2", target_bir_lowering=False, debug=False,
                   num_devices=NCORES)

    xh_d = nc.dram_tensor("xh", [TD, NSHP - WEX], F16,
                          kind="ExternalInput").ap()
    xp5_d = nc.dram_tensor("xp5", [TD, WEX, 2], F16,
                           kind="ExternalInput").ap()
    x32_d = nc.dram_tensor("x32", [NSH, TDP], F32, kind="ExternalInput").ap()
    whl_d = nc.dram_tensor("whl", [TDP, 2, GE], F16, kind="ExternalInput").ap()
    mats_d = nc.dram_tensor("mats", [GE, NMATS * GE], F32, kind="ExternalInput").ap()
    matsb_d = nc.dram_tensor("matsb", [GE, NMATS * GE], PDT, kind="ExternalInput").ap()
    bias_d = nc.dram_tensor("bias", [GE, NBIAS], F32, kind="ExternalInput").ap()
    biasb_d = nc.dram_tensor("biasb", [GE, NBIAS], PDT, kind="ExternalInput").ap()
    ltu_d = nc.dram_tensor("ltu", [128, 128], F16, kind="ExternalInput").ap()
    fixc_d = nc.dram_tensor("fixc", [128, NFIXC], F32, kind="ExternalInput").ap()
    out_d = nc.dram_tensor("out", [1, NSH], F32, kind="ExternalOutput").ap()

    def make_groups(kg):
        groups = []
        t = 0
        while t < KT:
            n = min(kg, KT - t)
            groups.append([t, n, False])
            t += n
        groups[-1][2] = True
        return groups

    with tile.TileContext(nc) as tc:
        nc.gpsimd.load_library(mlp)
        with (
            tc.tile_pool(name="consts", bufs=1) as consts,
            tc.tile_pool(name="xts", bufs=10) as xts,
            tc.tile_pool(name="work", bufs=1) as work,
            tc.tile_pool(name="ps", bufs=2, space="PSUM") as ptp,
        ):
            # ---- constants ----
            idt_sb = consts.tile([GE, GE], F32, tag="idt")
            matsb_sb = consts.tile([GE, NMATS * GE], PDT, tag="matsb")
            bias_sb = consts.tile([GE, NBIAS], F32, tag="bias")
            biasb_sb = consts.tile([GE, NBIAS], PDT, tag="biasb")
            ltu_sb = consts.tile([128, 128], F16, tag="ltu")
            fixc_sb = consts.tile([128, NFIXC], F32, tag="fixc")
            pred_sb = consts.tile([1, NSHP], F32, tag="pred")
            whl_sb = consts.tile([128, KB32, 2, GE], F16, tag="whl")

            def load_consts():
                nc.sync.dma_start(out=idt_sb,
                                  in_=mats_d[:, M_IDT * GE:(M_IDT + 1) * GE])
                nc.sync.dma_start(out=matsb_sb, in_=matsb_d)
                nc.sync.dma_start(out=bias_sb, in_=bias_d)
                nc.sync.dma_start(out=biasb_sb, in_=biasb_d)
                nc.sync.dma_start(out=ltu_sb, in_=ltu_d)
                nc.sync.dma_start(out=fixc_sb, in_=fixc_d)


            def load_wh_group(t0, n, tail):
                n2 = n + 1 if tail else n   # tail group also loads block 74
                nc.sync.dma_start(
                    out=whl_sb[:, t0:t0 + n2, :, :],
                    in_=whl_d[t0 * 128:(t0 + n2) * 128, :, :]
                        .rearrange("(t p) u m -> p t u m", p=128))

            def matb(i):
                return matsb_sb[:, i * GE:(i + 1) * GE]

            def bcol(i):
                return bias_sb[:, i:i + 1]

            idt = idt_sb
            ones128 = matsb_sb[:, M_ONES * GE:(M_ONES + 1) * GE]

            # ---------------- per-chunk post-processing ----------------
            def postA(cn, W_, rtr, sfx="", fix_c=None,
                      ttag="trp", tbufs=2):
                """Latency-critical head: h/h16/eh, transposed maxes, and
                (for main chunks) the flag->gather chain. Returns a context
                for postB."""
                nb = (W_ + 127) // 128
                pctx = dict(cn=cn, W_=W_, nb=nb, rtr=rtr, sfx=sfx)
                h_sb = work.tile([GE, W_], F32, tag="h" + sfx, bufs=2,
                                 name=f"h_{cn}")
                nc.scalar.activation(h_sb, rtr[:, :], AF.Identity,
                                     bias=bcol(B_BR), scale=1.0)
                h16 = work.tile([GE, W_], PDT, tag="h16" + sfx, bufs=2,
                                name=f"h16_{cn}")
                nc.scalar.activation(h16, rtr[:, :], AF.Identity,
                                     bias=bcol(B_BR), scale=1.0)
                eh = work.tile([GE, W_], F32, tag="eh" + sfx, bufs=2,
                               name=f"eh_{cn}")
                nc.scalar.activation(eh, rtr[:, :], AF.Exp,
                                     bias=bcol(B_BR), scale=1.0)
                pctx["h16"] = h16
                pctx["eh"] = eh

                # --- exact top-2 threshold (second max per row) ---
                trp = ptp.tile([128, nb, GE], F32, tag=ttag, bufs=tbufs,
                               name=f"trp_{cn}")
                cs_last = W_ - (nb - 1) * 128
                if cs_last < 128:
                    ms0 = (cs_last // 32) * 32
                    nc.vector.memset(trp[ms0:128, nb - 1, :], -1e30)
                for b in range(nb):
                    off = b * 128
                    cs = min(128, W_ - off)
                    nc.tensor.transpose(trp[:cs, b, :], h_sb[:, off:off + cs],
                                        idt)
                mx1 = work.tile([128, NB], F32, tag="mx1" + sfx,
                                name=f"mx1_{cn}")
                nc.vector.reduce_max(mx1[:, :nb], trp[:, :nb, :], axis=AX)
                mx2 = work.tile([128, NB], F32, tag="mx2" + sfx,
                                name=f"mx2_{cn}")
                mx1b = mx1[:, :nb].unsqueeze(2).to_broadcast([128, nb, GE])
                eqm = work.tile([128, NB, GE], F32, tag="eqm" + sfx,
                                name=f"eqm_{cn}")
                nc.vector.tensor_tensor(eqm[:, :nb, :], trp[:, :nb, :], mx1b,
                                        op=OP.is_ge)
                hm = work.tile([128, NB, GE], F32, tag="hm" + sfx,
                               name=f"hm_{cn}")
                nc.vector.scalar_tensor_tensor(
                    hm[:, :nb, :], in0=eqm[:, :nb, :], scalar=-1e30,
                    in1=trp[:, :nb, :], op0=OP.mult, op1=OP.add)
                nc.vector.reduce_max(mx2[:, :nb], hm[:, :nb, :], axis=AX)
                mx2b = mx2[:, :nb].unsqueeze(2).to_broadcast([128, nb, GE])
                pctx["trp"] = trp
                pctx["mx2"] = mx2

                if fix_c is not None:
                    pctx["fctx"] = fix_early(fix_c[0], fix_c[1], nb, trp,
                                             mx2, mx2b)
                return pctx

            def postB1(pctx, ptag="pp", stag="scr", pbufs=2, sbufs=1):
                """First half of the bulk post: mask, m1, q/k score chain."""
                cn, W_, nb = pctx["cn"], pctx["W_"], pctx["nb"]
                rtr, sfx = pctx["rtr"], pctx["sfx"]
                h16, eh, trp, mx2 = (pctx["h16"], pctx["eh"], pctx["trp"],
                                     pctx["mx2"])
                mx2b = mx2[:, :nb].unsqueeze(2).to_broadcast([128, nb, GE])

                # mask in transposed space, then PE-transpose back
                mkt = work.tile([128, NB, GE], F32, tag="mkt" + sfx,
                                name=f"mkt_{cn}")
                nc.vector.tensor_tensor(mkt[:, :nb, :], trp[:, :nb, :], mx2b,
                                        op=OP.is_ge)
                mask_ps = ptp.tile([GE, W_], F32, tag=ptag, bufs=pbufs,
                                   name=f"mask_{cn}")
                for b in range(nb):
                    off = b * 128
                    cs = min(128, W_ - off)
                    nc.tensor.transpose(mask_ps[:, off:off + cs],
                                        mkt[:cs, b, :], idt[:cs, :cs])
                m1 = work.tile([GE, W_], PDT, tag="m1" + sfx, name=f"m1_{cn}")
                nc.vector.tensor_mul(m1, eh, mask_ps[:, :])

                # --- attention (expert layer composed into Q/K/V) ---
                q_ps = ptp.tile([GE, W_], F32, tag=ptag, bufs=pbufs,
                                name=f"q_{cn}")
                nc.tensor.matmul(q_ps[:, :], lhsT=matb(M_AQ), rhs=h16,
                                 start=True, stop=True)
                qt = work.tile([GE, W_], F32, tag="qt" + sfx, name=f"qt_{cn}")
                nc.scalar.activation(qt, q_ps[:, :], AF.Identity,
                                     bias=bcol(B_BQ), scale=1.0)

                sc_ps = ptp.tile([GE, W_], F32, tag=stag, bufs=sbufs,
                                 name=f"sc_{cn}")
                for e in range(DH):
                    kr_ps = ptp.tile([GE, W_], F32, tag=ptag, bufs=pbufs,
                                     name=f"kr_{cn}_{e}")
                    nc.tensor.matmul(kr_ps[:, :], lhsT=matb(M_AK0 + e),
                                     rhs=h16, start=True, stop=True)
                    pe_sb = work.tile([GE, W_], PDT, tag="pe" + sfx, bufs=2,
                                      name=f"pe_{cn}_{e}")
                    nc.vector.scalar_tensor_tensor(
                        pe_sb, in0=kr_ps[:, :], scalar=bcol(B_BK0 + e),
                        in1=qt, op0=OP.add, op1=OP.mult)
                    nc.tensor.matmul(sc_ps[:, :], lhsT=matb(M_MS0 + e),
                                     rhs=pe_sb,
                                     start=(e == 0), stop=(e == DH - 1))
                es = work.tile([GE, W_], PDT, tag="es" + sfx, name=f"es_{cn}")
                nc.scalar.activation(es, sc_ps[:, :], AF.Exp, scale=0.5)

                den_ps = ptp.tile([GE, W_], F32, tag=ptag, bufs=pbufs,
                                  name=f"den_{cn}")
                nc.tensor.matmul(den_ps[:, :], lhsT=matb(M_MDEN), rhs=es,
                                 start=True, stop=True)
                w_ps = ptp.tile([GE, W_], F32, tag=ptag, bufs=pbufs,
                                name=f"w_{cn}")
                nc.tensor.matmul(w_ps[:, :], lhsT=matb(M_AOT), rhs=m1,
                                 start=True, stop=True)
                drec = work.tile([GE, W_], F32, tag="drec" + sfx,
                                 name=f"drec_{cn}")
                nc.vector.reciprocal(drec, den_ps[:, :])
                wd = work.tile([GE, W_], F32, tag="wd" + sfx, name=f"wd_{cn}")
                nc.vector.tensor_mul(wd, w_ps[:, :], drec)
                pctx["m1"] = m1
                pctx["es"] = es
                pctx["wd"] = wd

            def postB2(pctx, out_row, ptag="pp", stag="scr", pbufs=2):
                """Second half: value path, combine, final divide."""
                cn, W_, nb = pctx["cn"], pctx["W_"], pctx["nb"]
                sfx = pctx["sfx"]
                h16, eh = pctx["h16"], pctx["eh"]
                m1, es, wd = pctx["m1"], pctx["es"], pctx["wd"]
                prods = []
                for e in range(DH):
                    vr_ps = ptp.tile([GE, W_], F32, tag=ptag, bufs=pbufs,
                                     name=f"vr_{cn}_{e}")
                    nc.tensor.matmul(vr_ps[:, :], lhsT=matb(M_AV0 + e),
                                     rhs=h16, start=True, stop=True)
                    er_ps = ptp.tile([GE, W_], F32, tag=ptag, bufs=pbufs,
                                     name=f"er_{cn}_{e}")
                    nc.tensor.matmul(er_ps[:, :], lhsT=matb(M_MER0 + e),
                                     rhs=es, start=True, stop=True)
                    vrb = work.tile([GE, W_], F32, tag=f"vrb{e % 2}" + sfx,
                                    name=f"vrb_{cn}_{e}")
                    nc.scalar.activation(vrb, vr_ps[:, :], AF.Identity,
                                         bias=bcol(B_BV0 + e), scale=1.0)
                    pr = work.tile([GE, W_], PDT, tag=f"pr{e % 2}" + sfx,
                                   name=f"pr_{cn}_{e}")
                    nc.vector.tensor_mul(pr, er_ps[:, :], vrb)
                    prods.append(pr)
                t01 = work.tile([GE, W_], PDT, tag="t01" + sfx,
                                name=f"t01_{cn}")
                nc.vector.tensor_add(t01, prods[0], prods[1])
                t23 = work.tile([GE, W_], PDT, tag="t23" + sfx,
                                name=f"t23_{cn}")
                nc.vector.tensor_add(t23, prods[2], prods[3])
                att = work.tile([GE, W_], PDT, tag="att" + sfx,
                                name=f"att_{cn}")
                nc.vector.tensor_add(att, t01, t23)
                num = work.tile([GE, W_], PDT, tag="num" + sfx,
                                name=f"num_{cn}")
                nc.vector.tensor_mul(num, att, wd)

                # m1-fed matmul first: its operand is ready before num, and
                # the group's start must be the first matmul to execute
                nd1_ps = ptp.tile([1, W_], F32, tag=ptag, bufs=pbufs,
                                  name=f"nd1_{cn}")
                nc.tensor.matmul(nd1_ps[0:1, :],
                                 lhsT=biasb_sb[:, B_BO:B_BO + 1],
                                 rhs=m1, start=True, stop=False)
                nc.tensor.matmul(nd1_ps[0:1, :],
                                 lhsT=matsb_sb[:, M_ONES * GE:M_ONES * GE + 1],
                                 rhs=num, start=False, stop=True)
                nd2_ps = ptp.tile([1, W_], F32, tag=ptag, bufs=pbufs,
                                  name=f"nd2_{cn}")
                nc.tensor.matmul(nd2_ps[0:1, :],
                                 lhsT=matsb_sb[:, M_ONES * GE:M_ONES * GE + 1],
                                 rhs=m1, start=True, stop=True)
                rden = work.tile([1, W_], F32, tag="rden" + sfx,
                                 name=f"rden_{cn}")
                nc.vector.reciprocal(rden, nd2_ps[0:1, :])
                nc.vector.tensor_mul(out_row, nd1_ps[0:1, :], rden)

            # ---------------- fix-early: flags -> gather ----------------
            def fix_early(c, c0, nb, trp, mx2, mx2b):
                cn = f"c{c}"
                # flag: a third value within TAU of mx2, i.e.
                # count(trp + TAU >= mx2) >= 3
                eqm3 = work.tile([128, NB, GE], F32, tag="eqm3",
                                 name=f"eqm3_{cn}")
                nc.vector.scalar_tensor_tensor(
                    eqm3[:, :nb, :], in0=trp[:, :nb, :], scalar=TAU,
                    in1=mx2b, op0=OP.add, op1=OP.is_ge)
                cnt3 = work.tile([128, NB], F32, tag="cnt3",
                                 name=f"cnt3_{cn}")
                nc.vector.tensor_reduce(cnt3[:, :nb], eqm3[:, :nb, :],
                                        op=OP.add, axis=AX)
                flag0 = work.tile([128, NB], F32, tag="flag0",
                                  name=f"flag0_{cn}")
                nc.vector.tensor_scalar(flag0[:, :nb], cnt3[:, :nb],
                                        2.5, None, op0=OP.is_ge)
                flag = work.tile([128, NB], F16, tag="flag", bufs=2,
                                 name=f"flag_{cn}")
                nc.vector.tensor_tensor(
                    flag[:, :nb], flag0[:, :nb],
                    fixc_sb[:, FC_PM + c0 // 128:FC_PM + c0 // 128 + nb],
                    op=OP.mult)

                # rank: exclusive prefix-sum across the chunk's rows.
                # rank[:, b] = LTu^T flag[:, b] + ones^T cumflag[:, b] where
                # cumflag[:, b] = sum_{b'<b} flag[:, b'] (built by DVE).
                # Both matmuls write the same [128, nb] bytes, so the
                # accumulation group stays WAW-ordered under the scheduler.
                cumf = work.tile([128, NB], F16, tag="cumf",
                                 name=f"cumf_{cn}")
                nc.vector.memset(cumf[:, 0:1], 0.0)
                for b in range(1, nb):
                    nc.vector.tensor_add(cumf[:, b:b + 1],
                                         cumf[:, b - 1:b],
                                         flag[:, b - 1:b])
                rkix = ptp.tile([128, 8], F32, tag="fxp", bufs=1,
                                name=f"rkix_{cn}")
                nc.tensor.matmul(rkix[:, 0:nb], lhsT=ltu_sb,
                                 rhs=flag[:, 0:nb], start=True, stop=False)
                nc.tensor.matmul(rkix[:, 0:nb], lhsT=ones128,
                                 rhs=cumf[:, 0:nb], start=False, stop=True)

                oh = work.tile([128, NB, S], F32, tag="oh", bufs=2,
                               name=f"oh_{cn}")
                for b in range(nb):
                    rb = rkix[:, b:b + 1].to_broadcast([128, S])
                    fb = flag[:, b:b + 1].to_broadcast([128, S])
                    eqx = work.tile([128, S], F32, tag="eqx",
                                    name=f"eqx_{cn}_{b}")
                    nc.vector.tensor_tensor(eqx, rb, fixc_sb[:, FC_J:FC_J + S],
                                            op=OP.is_equal)
                    nc.vector.tensor_tensor(oh[:, b, :], eqx, fb, op=OP.mult)

                # idx extract with 8x-replicated onehot columns (stride-0
                # broadcast lhsT) -> [128, 1] directly in the 16-wrapped,
                # all-partition-replicated layout dma_gather wants
                gcol0 = c0 // 128
                for b in range(nb):
                    ohrep = work.tile([128, 8, S], F32, tag="ohrep",
                                      name=f"ohrep_{cn}_{b}")
                    nc.vector.tensor_copy(
                        ohrep, oh[:, b, :].unsqueeze(1)
                        .to_broadcast([128, 8, S]))
                    nc.tensor.matmul(rkix[:, 4:5], lhsT=ohrep,
                                     rhs=fixc_sb[:, FC_G + gcol0 + b:
                                                 FC_G + gcol0 + b + 1],
                                     start=(b == 0), stop=(b == nb - 1))
                idxw = work.tile([128, 1], I16, tag="idxw", bufs=2,
                                 name=f"idxw_{cn}")
                nc.vector.tensor_copy(idxw, rkix[:, 4:5])
                grows = work.tile([128, 1, TDP], F32, tag="grows",
                                  name=f"grows_{cn}")
                nc.gpsimd.dma_gather(grows[:, :, :], x32_d, idxw[:, :],
                                     S, S, TDP)
                return dict(c=c, c0=c0, oh=oh, grows=grows)

            # ------ fix-mid: gather->transpose->exact h into hfix_all ------
            NFIX = (len(CHUNKS) - 1) * S    # fix slots: all but last chunk
            hfix_all = ptp.tile([GE, NFIX], F32, tag="hfa", bufs=1,
                                name="hfix_all")

            def fix_mid(fctx):
                c, c0 = fctx["c"], fctx["c0"]
                Wc = fctx["W"]
                nbc = Wc // 128
                oh, grows = fctx["oh"], fctx["grows"]
                cn = f"f{c}"
                hfix = hfix_all[:, c * S:(c + 1) * S]
                first = True
                t0 = 0
                for bi, nbb in enumerate(FBATCH):
                    tp_ps = ptp.tile([128, nbb, S], F32, tag="fxp", bufs=1,
                                     name=f"tp_{cn}_{bi}")
                    for i in range(nbb):
                        blk = t0 + i
                        nc.tensor.transpose(
                            tp_ps[:, i, :],
                            grows[0:S, 0, blk * 128:(blk + 1) * 128],
                            idt[0:S, 0:S])
                    fh = work.tile([128, max(FBATCH), S], F16, tag="fh",
                                   bufs=2, name=f"fh_{cn}_{bi}")
                    nc.scalar.activation(fh[:, :nbb, :], tp_ps[:, :, :],
                                         AF.Identity, scale=1.0)
                    fl = work.tile([128, max(FBATCH), S], F16, tag="fl",
                                   bufs=2, name=f"fl_{cn}_{bi}")
                    nc.vector.scalar_tensor_tensor(
                        fl[:, :nbb, :], in0=tp_ps[:, :, :], scalar=1.0,
                        in1=fh[:, :nbb, :], op0=OP.mult, op1=OP.subtract)
                    for i in range(nbb):
                        blk = t0 + i
                        last = (bi == len(FBATCH) - 1 and i == nbb - 1)
                        nc.tensor.matmul(hfix[:, :], lhsT=whl_sb[:, blk, 0, :],
                                         rhs=fh[:, i, :],
                                         start=first, stop=False)
                        first = False
                        nc.tensor.matmul(hfix[:, :], lhsT=whl_sb[:, blk, 1, :],
                                         rhs=fh[:, i, :],
                                         start=False, stop=False)
                        nc.tensor.matmul(hfix[:, :], lhsT=whl_sb[:, blk, 0, :],
                                         rhs=fl[:, i, :],
                                         start=False, stop=last)
                    t0 += nbb
                # transpose oh now (overlapped); keep fp16 copy per chunk
                ohT_ps = ptp.tile([S, NB, 128], F32, tag="fxp", bufs=1,
                                  name=f"ohT_{cn}")
                for b in range(nbc):
                    nc.tensor.transpose(ohT_ps[:, b, :], oh[:, b, :], idt)
                ohT = work.tile([S, NB, 128], F16, tag="ohT",
                                bufs=len(CHUNKS), name=f"ohTs_{cn}")
                nc.scalar.activation(ohT[:, :nbc, :], ohT_ps[:, :nbc, :],
                                     AF.Identity, scale=1.0)
                fctx["ohT"] = ohT
                # mask does not depend on pfix: build + convert it here
                maskf_ps = ptp.tile([1, WMAX], F32, tag="pp", bufs=2,
                                    name=f"maskf_{cn}")
                for b in range(nbc):
                    nc.tensor.matmul(maskf_ps[:, b * 128:(b + 1) * 128],
                                     lhsT=matsb_sb[0:S, M_ONES * GE:
                                                   M_ONES * GE + 1],
                                     rhs=ohT[:, b, :],
                                     start=True, stop=True)
                masku8 = work.tile([1, WMAX], U8, tag="masku8",
                                   bufs=len(CHUNKS), name=f"masku8_{cn}")
                nc.scalar.activation(masku8[:, :Wc], maskf_ps[:, :Wc],
                                     AF.Identity, scale=1.0)
                fctx["masku8"] = masku8

            # ------ fix-tail: combined post over a chunk group + blends ------
            def fix_tail(fctxs):
                cg = f"g{fctxs[0]['c']}"
                nfx = len(fctxs) * S
                cb = fctxs[0]["c"] * S
                pfix_all = work.tile([1, NFIX], F32, tag="pfixall", bufs=2,
                                     name=f"pfa_{cg}")
                fp = postA(f"fall_{cg}", nfx, hfix_all[:, cb:cb + nfx],
                           sfx="f", ttag="trp", tbufs=2)
                postB1(fp)
                postB2(fp, pfix_all[0:1, :nfx])
                pfx16s = []
                for fi in range(len(fctxs)):
                    pT = ptp.tile([S, 1], F32, tag="fxp", bufs=1,
                                  name=f"pfxT_{cg}_{fi}")
                    nc.tensor.transpose(pT[:, :],
                                        pfix_all[0:1, fi * S:(fi + 1) * S],
                                        idt[0:1, 0:1])
                    p16 = work.tile([S, 1], F16, tag="pfx16",
                                    bufs=len(CHUNKS), name=f"pfx16_{cg}_{fi}")
                    nc.scalar.activation(p16, pT[:, :], AF.Identity,
                                         scale=1.0)
                    pfx16s.append(p16)
                for fi, fctx in enumerate(fctxs):
                    c, c0, Wc = fctx["c"], fctx["c0"], fctx["W"]
                    nbc = Wc // 128
                    ohT = fctx["ohT"]
                    masku8 = fctx["masku8"]
                    cn = f"t{c}"
                    # per-block single-matmul groups (atomic wrt the zero
                    # region) -- the scheduler may reorder across blocks
                    scat_ps = ptp.tile([1, WMAX], F32, tag="pp", bufs=2,
                                       name=f"scat_{cn}")
                    for b in range(nbc):
                        nc.tensor.matmul(scat_ps[:, b * 128:(b + 1) * 128],
                                         lhsT=pfx16s[fi], rhs=ohT[:, b, :],
                                         start=True, stop=True)
                    nc.vector.copy_predicated(pred_sb[0:1, c0:c0 + Wc],
                                              masku8[:, :Wc],
                                              scat_ps[:, :Wc])

            # ---------------- main pipeline ----------------
            fixq = []
            pq = []
            c0 = 0
            nch = len(CHUNKS)
            for c, Wc in enumerate(CHUNKS):
                exact = (c == nch - 1)
                rtr = ptp.tile([GE, Wc], F32, tag="rtr", bufs=1,
                               name=f"rtr_c{c}")
                first = True
                groups = make_groups(KG)
                for gi, (t0, ntl, tail) in enumerate(groups):
                    if c == 0 and gi == 1:
                        load_consts()
                    if c == 0:
                        load_wh_group(t0, ntl, tail)
                    if not exact:
                        xb = xts.tile([128, KG + 1, WMAX], F16, tag="xb",
                                      name=f"xb_c{c}_g{gi}")
                        nc.sync.dma_start(
                            out=xb[:, :ntl, :Wc],
                            in_=xh_d[t0 * 128:(t0 + ntl) * 128, c0:c0 + Wc]
                                .rearrange("(t p) m -> p t m", p=128))
                        if tail:
                            nc.sync.dma_start(
                                out=xb[:KTAIL, ntl, :Wc],
                                in_=xh_d[KT * 128:TD, c0:c0 + Wc])
                        for i in range(ntl):
                            t = t0 + i
                            nc.tensor.matmul(rtr[:, :],
                                             lhsT=whl_sb[:, t, 0, :],
                                             rhs=xb[:, i, :Wc],
                                             start=first, stop=False)
                            first = False
                        if tail:
                            nc.tensor.matmul(rtr[:, :],
                                             lhsT=whl_sb[0:KTAIL, KT, 0, :],
                                             rhs=xb[:KTAIL, ntl, :Wc],
                                             start=False, stop=True)
                    else:
                        # exact fp16-pair stream (hi@wh + hi@wl + lo@wh)
                        xb = xts.tile([128, KG + 1, WEX, 2], F16, tag="xb",
                                      name=f"xb_c{c}_g{gi}")
                        nc.sync.dma_start(
                            out=xb[:, :ntl, :, :],
                            in_=xp5_d[t0 * 128:(t0 + ntl) * 128, :, :]
                                .rearrange("(t p) m u -> p t m u", p=128))
                        if tail:
                            nc.sync.dma_start(
                                out=xb[:KTAIL, ntl, :, :],
                                in_=xp5_d[KT * 128:TD, :, :])
                        for i in range(ntl):
                            t = t0 + i
                            nc.tensor.matmul(rtr[:, :],
                                             lhsT=whl_sb[:, t, 0, :],
                                             rhs=xb[:, i, :, 0],
                                             start=first, stop=False)
                            first = False
                            nc.tensor.matmul(rtr[:, :],
                                             lhsT=whl_sb[:, t, 1, :],
                                             rhs=xb[:, i, :, 0],
                                             start=False, stop=False)
                            nc.tensor.matmul(rtr[:, :],
                                             lhsT=whl_sb[:, t, 0, :],
                                             rhs=xb[:, i, :, 1],
                                             start=False, stop=False)
                        if tail:
                            nc.tensor.matmul(rtr[:, :],
                                             lhsT=whl_sb[0:KTAIL, KT, 0, :],
                                             rhs=xb[:KTAIL, ntl, :, 0],
                                             start=False, stop=False)
                            nc.tensor.matmul(rtr[:, :],
                                             lhsT=whl_sb[0:KTAIL, KT, 1, :],
                                             rhs=xb[:KTAIL, ntl, :, 0],
                                             start=False, stop=False)
                            nc.tensor.matmul(rtr[:, :],
                                             lhsT=whl_sb[0:KTAIL, KT, 0, :],
                                             rhs=xb[:KTAIL, ntl, :, 1],
                                             start=False, stop=True)
                if pq:
                    postB1(pq[-1]["pctx"])
                if fixq:
                    fix_mid(fixq[-1])
                if exact:
                    fix_tail(fixq)        # fix post runs under this stream
                pctx = postA(f"c{c}", Wc, rtr, sfx="",
                             fix_c=None if exact else (c, c0))
                if not exact:
                    fctx = pctx["fctx"]
                    fctx["W"] = Wc
                    fctx["pctx"] = dict(pctx=pctx, c0=c0, W=Wc)
                    fixq.append(fctx)
                if pq:
                    prev = pq.pop()
                    postB2(prev["pctx"], pred_sb[0:1, prev["c0"]:
                                                 prev["c0"] + prev["W"]])
                pq.append(dict(pctx=pctx, c0=c0, W=Wc))
                c0 += Wc
            prev = pq.pop()
            postB1(prev["pctx"])
            postB2(prev["pctx"], pred_sb[0:1, prev["c0"]:
                                         prev["c0"] + prev["W"]])
            nc.sync.dma_start(out=out_d, in_=pred_sb[:, 0:NSH])

    nc.compile()
    return nc


_NC_CACHE = None
LAST_RESULTS = None


def kernel(x, Wr, br, We, be, Wq, bq, Wk, bk, Wv, bv, Wo, bo):
    global _NC_CACHE, LAST_RESULTS
    if _NC_CACHE is None:
        _NC_CACHE = build_kernel()
    nc = _NC_CACHE

    in_maps = core_inputs(x, Wr, br, We, be, Wq, bq, Wk, bk, Wv, bv, Wo, bo)
    res = run_bass_kernel_spmd(nc, in_maps, list(range(NCORES)))
    LAST_RESULTS = res
    out = np.concatenate([res.results[c]["out"].reshape(NSH)
                          for c in range(NCORES)])
    return out.astype(np.float32)
